# revision 50
# baseline (speedup 1.0000x reference)
"""CrossCLR intra-modality loss on 8 Trainium2 NeuronCores — v4.

Data-parallel over the 4096-row batch (512 rows/core), fp8-e4m3
normalized embeddings, DoubleRow matmuls. Structure:

* Column rotation: per-core inputs are host-rolled by c*512 columns, so
  each core's slab sits at columns [0, 512) and gram diagonals at the
  fixed position col = mt*128 + p. One SPMD instruction stream.
* Symmetric gram halving: the brand gram C and post gram D are
  symmetric; each core computes only rotated cols [0, 2560); row-sum
  contributions of distance 5..7 blocks come from partners' column
  sums over [512, 2048), exchanged through the host.
* Diag masking on PE: a [128, 512] identity-matmul accumulates -16
  onto the diag strip in PSUM; exp underflows to exactly 0 there and
  the reference's exp(0)=1 diag is restored as +1.0 on the host.
* Merged pipeline: C, G(=post_l @ brand^T), and D tiles interleave
  (PSUM 4+4 banks); junk warmup matmuls finish the PE p-state ramp
  during the DMA fill. ACT work per CD unit (~1.4us) matches DVE work
  per G unit (~2.4us) at a 1:1 interleave.
* No collective, no on-device partition reductions: fp8 exp(cos/T-ln2)
  tiles and paired bf16 column-count indicator tiles are DMA'd out;
  the HOST does partition + cross-core sums and the O(N) epilogue. G
  row sums come from the same fp8 export (no ACT accumulator reads).
* Export tiles live in LARGE rings (ex 16, ia 4, ind 6): export DMAs
  drain behind the input loads on the serial DMA engine, and a small
  ring would stall ACT/Pool on export completion (WAR), freezing the
  whole G pipeline — this was a measured 9.5us stall.
"""

import sys

sys.path.insert(0, "/opt/trn_rl_repo")

import math
from contextlib import ExitStack
from functools import lru_cache

import ml_dtypes
import numpy as np

import concourse.bacc as bacc
import concourse.mybir as mybir
import concourse.tile as tile
from concourse.bass_utils import run_bass_kernel_spmd

N = 4096
D = 1024
NC = 8
S = N // NC
P = 128
KC = D // P
MT = S // P
SEG = 512
SEGW = 1024
NSEGW = N // SEGW
CW = 2560
CS0, CS1 = 512, 2048
CWIN = CS1 - CS0
TEMP = 0.03
NEG_W = 0.8
BNEG = -16.0
LN2 = math.log(2.0)
MW = 896 + P + MT + MT + SEGW + SEGW

F32 = mybir.dt.float32
BF16 = mybir.dt.bfloat16
FP8 = mybir.dt.float8e4
AF = mybir.ActivationFunctionType
OP = mybir.AluOpType
PM = mybir.MatmulPerfMode.DoubleRow
E4M3 = ml_dtypes.float8_e4m3
BF16_NP = ml_dtypes.bfloat16


def build_program():
    nc = bacc.Bacc("TRN2", target_bir_lowering=False, debug=False, num_devices=NC)

    bTn_d = nc.dram_tensor("bTn", (D, N), FP8, kind="ExternalInput")
    pTn_d = nc.dram_tensor("pTn", (D, CW), FP8, kind="ExternalInput")
    misc_d = nc.dram_tensor("misc", (P, MW), BF16, kind="ExternalInput")
    trow_d = nc.dram_tensor("trow", (1, N), BF16, kind="ExternalInput")
    invB_d = nc.dram_tensor("invB", (1, N), BF16, kind="ExternalInput")
    outA_d = nc.dram_tensor("outA", (P, 3 * MT), F32, kind="ExternalOutput")
    # per sw: [ex_mt0..ex_mt3] x P rows each, rotated cols
    gcol_d = nc.dram_tensor("gcol", (NSEGW * 4 * P, SEGW), FP8,
                            kind="ExternalOutput")
    # per sw: [ind01, ind23] x P rows
    gind_d = nc.dram_tensor("gind", (NSEGW * 2 * P, SEGW), BF16,
                            kind="ExternalOutput")
    csum_d = nc.dram_tensor("csum", (2 * P, CWIN), BF16, kind="ExternalOutput")

    with tile.TileContext(nc) as tc, ExitStack() as ctx:
        pin = ctx.enter_context(tc.tile_pool(name="pin", bufs=1))
        pstat = ctx.enter_context(tc.tile_pool(name="pstat", bufs=1))
        pex = ctx.enter_context(tc.tile_pool(name="pex", bufs=16))
        pexcd = ctx.enter_context(tc.tile_pool(name="pexcd", bufs=8))
        pind = ctx.enter_context(tc.tile_pool(name="pind", bufs=10))
        pacc = ctx.enter_context(tc.tile_pool(name="pacc", bufs=4))
        pjunk = ctx.enter_context(tc.tile_pool(name="pjunk", bufs=4))

        def load_cols(eng, dst, src_d, c0, c1):
            eng.dma_start(out=dst[:, :, c0:c1],
                          in_=src_d.ap()[:, c0:c1]
                          .rearrange("(k p) n -> p k n", p=P))

        def bcast(dst, src_d, c0, c1):
            nc.sync.dma_start(out=dst[:, c0:c1],
                              in_=src_d.ap()[0:1, c0:c1].partition_broadcast(P))

        ones = pin.tile([P, 1], BF16, tag="ones")
        nc.vector.memset(ones[:], 1.0)
        lnb = pin.tile([P, 1], F32, tag="lnb")
        nc.vector.memset(lnb[:], -LN2)
        warm = pin.tile([P, SEG], BF16, tag="warm")
        nc.gpsimd.memset(warm[:], 0.0)
        with tc.tile_pool(name="pwarm", bufs=1, space="PSUM") as pwarm:
            wps = pwarm.tile([1, SEG], F32, tag="wps")
            for _ in range(11):
                nc.tensor.matmul(wps[:], ones[:], warm[:],
                                 start=True, stop=True)

        bTn = pin.tile([P, KC, N], FP8, tag="bTn")
        pTn = pin.tile([P, KC, CW], FP8, tag="pTn")
        trow = pin.tile([P, N], BF16, tag="trow")
        invB = pin.tile([P, N], BF16, tag="invB")
        misc = pin.tile([P, MW], BF16, tag="misc")
        dmask = misc[:, 0:896]
        ident = misc[:, 896:896 + P]
        ci = misc[:, 896 + P:896 + P + MT]
        invPl = misc[:, 896 + P + MT:896 + P + 2 * MT]
        trow0 = misc[:, 896 + P + 2 * MT:896 + P + 2 * MT + SEGW]
        invB0 = misc[:, 896 + P + 2 * MT + SEGW:MW]

        load_cols(nc.sync, bTn, bTn_d, 0, SEG)
        load_cols(nc.sync, bTn, bTn_d, SEG, SEGW)
        load_cols(nc.sync, pTn, pTn_d, 0, S)
        nc.sync.dma_start(out=misc[:], in_=misc_d[:, :])
        load_cols(nc.sync, bTn, bTn_d, SEGW, 2 * SEGW)
        bcast(trow, trow_d, SEGW, 2 * SEGW)
        bcast(invB, invB_d, SEGW, 2 * SEGW)
        load_cols(nc.sync, bTn, bTn_d, 2 * SEGW, CW)
        load_cols(nc.sync, bTn, bTn_d, CW, 3 * SEGW)
        bcast(trow, trow_d, 2 * SEGW, 3 * SEGW)
        bcast(invB, invB_d, 2 * SEGW, 3 * SEGW)
        load_cols(nc.sync, pTn, pTn_d, S, 3 * SEG)
        load_cols(nc.sync, bTn, bTn_d, 3 * SEGW, 3 * SEGW + SEG)
        load_cols(nc.sync, bTn, bTn_d, 3 * SEGW + SEG, N)
        bcast(trow, trow_d, 3 * SEGW, N)
        bcast(invB, invB_d, 3 * SEGW, N)
        load_cols(nc.sync, pTn, pTn_d, 3 * SEG, 2 * SEGW)
        load_cols(nc.sync, pTn, pTn_d, 2 * SEGW, CW)

        cntP = pstat.tile([P, MT, NSEGW], F32, tag="cntP")
        sC, sD = (pstat.tile([P, MT, 3], F32, tag=n, name=n)
                  for n in ("sC", "sD"))
        csumC = pstat.tile([P, CWIN], BF16, tag="csumC")
        csumD = pstat.tile([P, CWIN], BF16, tag="csumD")

        def mm(ps, lhsT, rhs, mt, col0, width, open_segs=()):
            for h in range(width // SEG):
                for kk in range(KC // 2):
                    nc.tensor.matmul(
                        ps[:, h * SEG:(h + 1) * SEG],
                        lhsT[:, 2 * kk:2 * kk + 2, mt * P:(mt + 1) * P],
                        rhs[:, 2 * kk:2 * kk + 2,
                            col0 + h * SEG:col0 + (h + 1) * SEG],
                        start=(kk == 0),
                        stop=(kk == KC // 2 - 1 and h not in open_segs),
                        perf_mode=PM)

        cd_state = {}

        def cd_unit(which, mt, ch, pcd):
            src = bTn if which == "C" else pTn
            slots = sC if which == "C" else sD
            csum = csumC if which == "C" else csumD
            st = cd_state.setdefault(which, {"exs": {}, "p01": {}})
            width = SEG if ch == 2 else SEGW
            ps = pcd.tile([P, SEGW], F32, tag="cd", name="cd")
            mm(ps, src, src, mt, ch * SEGW, width,
               open_segs=(0,) if ch == 0 else ())
            if ch == 0:
                dm = dmask[:, 384 - mt * P:384 - mt * P + SEG]
                nc.tensor.matmul(ps[:, 0:SEG], ident[:], dm,
                                 start=False, stop=True)
            if ch == 2:
                junk = pjunk.tile([P, SEG], BF16, tag="junkcd", name="junkcd")
                nc.scalar.activation(out=junk[:], in_=ps[:, 0:SEG],
                                     func=AF.Exp, scale=NEG_W / TEMP,
                                     accum_out=slots[:, mt, ch:ch + 1])
                return
            ex = pexcd.tile([P, SEGW], BF16, tag="excd", name="excd")
            nc.scalar.activation(out=ex[:], in_=ps[:], func=AF.Exp,
                                 scale=NEG_W / TEMP,
                                 accum_out=slots[:, mt, ch:ch + 1])
            # C's colsum adds ride the idle GPSIMD; D's run near the tail
            # where GPSIMD's slow adds would gate the csum export -> DVE.
            w = slice(SEG, SEGW) if ch == 0 else slice(0, SEGW)
            ww = SEG if ch == 0 else SEGW
            co = 0 if ch == 0 else SEG
            eng = nc.gpsimd if which == "C" else nc.vector
            if mt % 2 == 0:
                st["exs"][ch] = ex
            else:
                prev = st["exs"].pop(ch)
                if mt == 1:
                    p01 = pacc.tile([P, ww], BF16, tag=f"cd01_{ch}",
                                    name="p01")
                    eng.tensor_tensor(out=p01[:], in0=prev[:, w],
                                      in1=ex[:, w], op=OP.add)
                    st["p01"][ch] = p01
                else:
                    p23 = pacc.tile([P, ww], BF16, tag=f"cd23_{ch}",
                                    name="p23")
                    eng.tensor_tensor(out=p23[:], in0=prev[:, w],
                                      in1=ex[:, w], op=OP.add)
                    p01 = st["p01"].pop(ch)
                    eng.tensor_tensor(
                        out=csum[:, co:co + ww], in0=p01[:], in1=p23[:],
                        op=OP.add)

        g_state = {"inds": []}

        def g_unit(sw, mt, pg):
            tr = trow0 if sw == 0 else trow[:, sw * SEGW:(sw + 1) * SEGW]
            ib = invB0 if sw == 0 else invB[:, sw * SEGW:(sw + 1) * SEGW]
            ps = pg.tile([P, SEGW], F32, tag="g", name="g")
            mm(ps, pTn, bTn, mt, sw * SEGW, SEGW)
            # fp8 export tile: exp(cos/T - ln2); host doubles the sums and
            # derives BOTH row and column exp sums from it.
            ex = pex.tile([P, SEGW], FP8, tag="ex", name="ex")
            nc.scalar.activation(out=ex[:], in_=ps[:], func=AF.Exp,
                                 scale=1.0 / TEMP, bias=lnb[:, 0:1])
            base = (sw * 4 + mt) * P
            nc.sync.dma_start(out=gcol_d[base:base + P, :], in_=ex[:])
            ind = pind.tile([P, SEGW], BF16, tag="ind", name="ind")
            nc.vector.scalar_tensor_tensor(
                out=ind[:], in0=tr, scalar=invPl[:, mt:mt + 1], in1=ps[:],
                op0=OP.mult, op1=OP.is_lt)
            junk = pjunk.tile([P, SEGW], BF16, tag="junk", name="junk")
            nc.vector.scalar_tensor_tensor(
                out=junk[:], in0=ib, scalar=ci[:, mt:mt + 1], in1=ps[:],
                op0=OP.mult, op1=OP.is_lt,
                accum_out=cntP[:, mt, sw:sw + 1])
            g_state["inds"].append(ind)
            if mt % 2 == 1:
                k = (mt - 1) // 2
                inds = g_state["inds"]
                ia = pacc.tile([P, SEGW], BF16, tag=f"ia{k}", name="ia")
                nc.gpsimd.tensor_tensor(out=ia[:], in0=inds[-2][:],
                                        in1=inds[-1][:], op=OP.add)
                base = (sw * 2 + k) * P
                nc.sync.dma_start(out=gind_d[base:base + P, :], in_=ia[:])
                if mt == 3:
                    g_state["inds"] = []

        outA = pstat.tile([P, 3 * MT], F32, tag="outA")
        C, D_ = "C", "D"
        cd_seq = ([(C, mt, 0) for mt in range(MT)]
                  + [(C, mt, 1) for mt in range(MT)]
                  + [(C, mt, 2) for mt in range(MT)]
                  + [(D_, mt, 0) for mt in range(MT)]
                  + [(D_, mt, 1) for mt in range(MT)]
                  + [(D_, mt, 2) for mt in range(MT)])
        g_seq = [("G", sw, mt) for sw in range(NSEGW) for mt in range(MT)]
        g_pos = [2, 4, 6, 8, 10, 12, 14, 16, 18, 20, 22, 24, 26, 28, 30, 32]
        sched = []
        i_cd = i_g = 0
        for slot in range(40):
            if i_g < 16 and slot == g_pos[i_g]:
                sched.append(g_seq[i_g])
                i_g += 1
            else:
                u = cd_seq[i_cd]
                i_cd += 1
                sched.append(u)
                if u == (C, 3, 1):
                    sched.append(("csumC",))
                elif u == (C, 3, 2):
                    sched.append(("redC",))
                elif u == (D_, 3, 1):
                    sched.append(("csumD",))
        sched.append(("redG",))

        with tc.tile_pool(name="pcd", bufs=2, space="PSUM") as pcd, \
             tc.tile_pool(name="pg", bufs=2, space="PSUM") as pg:
            for u in sched:
                if u[0] == "G":
                    g_unit(u[1], u[2], pg)
                elif u[0] == "csumC":
                    nc.sync.dma_start(out=csum_d[0:P, :], in_=csumC[:])
                elif u[0] == "csumD":
                    nc.sync.dma_start(out=csum_d[P:2 * P, :], in_=csumD[:])
                elif u[0] == "redC":
                    nc.vector.tensor_reduce(outA[:, MT:2 * MT], sC[:, :, :],
                                            mybir.AxisListType.X, OP.add)
                elif u[0] == "redG":
                    nc.vector.tensor_reduce(outA[:, 0:MT], cntP[:, :, :],
                                            mybir.AxisListType.X, OP.add)
                else:
                    cd_unit(u[0], u[1], u[2], pcd)

        nc.vector.tensor_reduce(outA[:, 2 * MT:3 * MT], sD[:, :, :],
                                mybir.AxisListType.X, OP.add)
        nc.sync.dma_start(out=outA_d[:, :], in_=outA[:])

    nc.compile()
    return nc


@lru_cache(maxsize=1)
def _program():
    return build_program()


def _core_inputs(brand, post):
    nB = np.linalg.norm(brand, axis=1)
    nP = np.linalg.norm(post, axis=1)
    bTn = np.ascontiguousarray((brand / nB[:, None]).T).astype(E4M3)
    pTn = np.ascontiguousarray((post / nP[:, None]).T).astype(E4M3)
    diag_full = (bTn.astype(np.float32) * pTn.astype(np.float32)).sum(0)
    trow = (diag_full * nP).astype(np.float32)
    invB = (1.0 / nB).astype(np.float32)
    ident = np.eye(P, dtype=BF16_NP)
    dmask = np.zeros((P, 896), dtype=BF16_NP)
    dmask[np.arange(P), 384 + np.arange(P)] = BNEG
    maps = []
    for c in range(NC):
        rot = c * S
        sl = slice(rot, rot + S)
        trow_r = np.roll(trow, -rot).astype(BF16_NP)
        invB_r = np.roll(invB, -rot).astype(BF16_NP)
        misc = np.concatenate([
            dmask,
            ident,
            (diag_full[sl] * nB[sl]).reshape(MT, P).T.astype(BF16_NP),
            (1.0 / nP[sl]).reshape(MT, P).T.astype(BF16_NP),
            np.tile(trow_r[0:SEGW], (P, 1)),
            np.tile(invB_r[0:SEGW], (P, 1)),
        ], axis=1)
        maps.append({
            "bTn": np.ascontiguousarray(np.roll(bTn, -rot, axis=1)),
            "pTn": np.ascontiguousarray(np.roll(pTn, -rot, axis=1)[:, :CW]),
            "misc": np.ascontiguousarray(misc),
            "trow": trow_r.reshape(1, N),
            "invB": invB_r.reshape(1, N),
        })
    return maps, diag_full


def kernel(brand, post):
    brand = np.asarray(brand, dtype=np.float32)
    post = np.asarray(post, dtype=np.float32)
    nc = _program()
    maps, diag_full = _core_inputs(brand, post)
    res = run_bass_kernel_spmd(nc, maps, list(range(NC)))

    cnt_post = np.zeros(N)
    sG = np.zeros(N)
    sC = np.zeros(N)
    sD = np.zeros(N)
    colcnt = np.zeros(N)
    colexp = np.zeros(N)
    csumC = np.zeros(N)
    csumD = np.zeros(N)
    for c, r in enumerate(res.results):
        rot = c * S
        sl = slice(rot, rot + S)
        oa = r["outA"].astype(np.float64)

        def unslot(x):
            return x.T.reshape(S)

        cnt_post[sl] = unslot(oa[:, 0:MT])
        sC[sl] = unslot(oa[:, MT:2 * MT])
        sD[sl] = unslot(oa[:, 2 * MT:3 * MT])
        gi = r["gind"].astype(np.float64).reshape(NSEGW, 2, P, SEGW)
        ccnt = (gi[:, 0] + gi[:, 1]).sum(1).reshape(N)
        colcnt += np.roll(ccnt, rot)
        exs = r["gcol"].astype(np.float64).reshape(NSEGW, 4, P, SEGW)
        colexp += np.roll(2.0 * exs.sum((1, 2)).reshape(N), rot)
        sG[sl] = 2.0 * exs.sum((0, 3)).reshape(S)
        cs = r["csum"].astype(np.float64)
        t = np.zeros(N)
        t[CS0:CS1] = cs[0:P].sum(0)
        csumC += np.roll(t, rot)
        t = np.zeros(N)
        t[CS0:CS1] = cs[P:2 * P].sum(0)
        csumD += np.roll(t, rot)

    d64 = diag_full.astype(np.float64)
    es_post = sG + sD + csumD + 1.0
    es_brand = colexp + sC + csumC + 1.0
    loss_p = (1.0 + 1.0 / (N - cnt_post)) * (np.log(es_post) - d64 / TEMP)
    loss_b = (1.0 + 1.0 / (N - colcnt)) * (np.log(es_brand) - d64 / TEMP)
    return np.float32((loss_b.sum() + loss_p.sum()) / 2.0)
